# revision 12
# baseline (speedup 1.0000x reference)
"""Trainium2 Bass kernel v2 for the dense transformer block (B=2,T=2048,C=1024,H=16).

Sharding: token-parallel, same-batch causal grouping. Core c = (b=c//4, j=c%4)
handles batch b, query blocks {j, 7-j} of 256 rows each. Attention runs in two
groups per core:
  group0: queries block j    -> keys: own 256 (diag, 2 kt) + prefix 0..2j*128 (pad 6 kt)
  group1: queries block 7-j  -> keys: own 256 (diag, 2 kt) + prefix (pad 14 kt)
Diagonal keys come first (static triangular mask); prefix tiles are masked
real/pad by a per-core additive exp bias (ln16 for real pairs, -1e9 for pad),
constant across each fused 2-kt exp instruction (real/pad boundary 2j is even).

fp8 (e4m3) with DoubleRow matmuls for: main K/V projections (weights x64,
dequant 1/64 fused into the PSUM->SBUF move) and AV (p = 16*exp via ln16 bias,
v x8 via the V move scale; the 128x factor cancels in y/denominator).
Everything else bf16 with fp32 PSUM. LayerNorm affine folded into downstream
weights host-side; bv folded into bo. Reciprocals via exp(-ln(x)) on ACT.
"""

import math
from contextlib import ExitStack

import numpy as np
import ml_dtypes

import concourse.bass as bass
import concourse.mybir as mybir
import concourse.tile as tile
from concourse import bacc
from concourse.bass_utils import run_bass_kernel_spmd


F32 = mybir.dt.float32
BF16 = mybir.dt.bfloat16
FP8 = mybir.dt.float8e4
FP = mybir.AluOpType
AF = mybir.ActivationFunctionType
DR = mybir.MatmulPerfMode.DoubleRow

B, T, C, H, D, FF = 2, 2048, 1024, 16, 64, 4096
NCORES = 8
P = 128
CH = C // P              # 8 feature chunks
FH = FF // P             # 32 ff chunks
QC = 256                 # queries per block
NQ = 2 * QC              # 512 queries per core
NKM = 1792               # main (prefix) keys per core, static
NKMT = NKM // P          # 14 main key tiles
GPAIRS = (3, 7)          # fused prefix kt-pairs per group
NPAIR_BIAS = 2 + GPAIRS[0] + GPAIRS[1]   # 12 kbias entries (diag g0, p0..2, diag g1, p0..6)
NEG = -1.0e9
SM_SCALE = 1.0 / math.sqrt(D)
WS = 64.0                # fp8 weight scale
PB = -2.0                # p exp bias: p = exp(s/8 - 2), keeps p in e4m3 range
VS8 = 8.0                # v fp8 scale (cancels with exp bias in y/den)

_NC_CACHE = {}


def _build_nc(reps=1, stop_after=None):
    nc = bacc.Bacc("TRN2", target_bir_lowering=False, debug=False)

    # ---- DRAM I/O ----
    xm = nc.dram_tensor("xm", [C, NKM], BF16, kind="ExternalInput")
    xq = nc.dram_tensor("xq", [C, NQ], BF16, kind="ExternalInput")
    xqf = nc.dram_tensor("xqf", [C, NQ], F32, kind="ExternalInput")
    wq = nc.dram_tensor("wq", [CH, P, C], BF16, kind="ExternalInput")
    wo = nc.dram_tensor("wo", [CH, P, C], BF16, kind="ExternalInput")
    w1 = nc.dram_tensor("w1", [FH, P, C], BF16, kind="ExternalInput")
    w2 = nc.dram_tensor("w2", [CH, P, FF], FP8, kind="ExternalInput")  # x64
    wk8 = nc.dram_tensor("wk8", [CH, P, C], FP8, kind="ExternalInput")   # x64
    wv8 = nc.dram_tensor("wv8", [P, CH, C], FP8, kind="ExternalInput")   # x64, natural rhs
    bias = nc.dram_tensor("bias", [P, 4 * CH], F32, kind="ExternalInput")
    b1d = nc.dram_tensor("b1", [P, FH], F32, kind="ExternalInput")
    kbias = nc.dram_tensor("kbias", [P, NPAIR_BIAS], F32, kind="ExternalInput")
    yt = nc.dram_tensor("yt", [C, NQ], F32, kind="ExternalOutput")

    xm_r = xm[:].rearrange("(a p) n -> p a n", p=P)    # [128, 8, 1792]
    xq_r = xq[:].rearrange("(a p) n -> p a n", p=P)    # [128, 8, 512]
    xqf_r = xqf[:].rearrange("(a p) n -> p a n", p=P)
    yt_r = yt[:].rearrange("(a p) n -> p a n", p=P)

    with tile.TileContext(nc) as tc, ExitStack() as E:
        consts = E.enter_context(tc.tile_pool(name="consts", bufs=1))
        bias_sb = consts.tile([P, 4 * CH], F32)
        nc.sync.dma_start(out=bias_sb, in_=bias[:])
        bq_s = bias_sb[:, 0:8]
        bk_s = bias_sb[:, 8:16]
        bo_s = bias_sb[:, 16:24]
        b2_s = bias_sb[:, 24:32]

        b1_sb = consts.tile([P, FH], F32)
        nc.sync.dma_start(out=b1_sb, in_=b1d[:])
        kb_sb = consts.tile([P, NPAIR_BIAS], F32)
        nc.sync.dma_start(out=kb_sb, in_=kbias[:])

        ones_bf = consts.tile([P, 1], BF16)
        nc.vector.memset(ones_bf, 1.0)
        eps_sb = consts.tile([1, 1], F32)
        nc.vector.memset(eps_sb, 1e-5)

        # constant diag mask [128, 2kt, 256]: kt0 pass q>=k ; kt1 pass q>=k+128
        qmask = consts.tile([P, 2, QC], BF16)
        nc.gpsimd.memset(qmask, 1.0)
        for kt in range(2):
            nc.gpsimd.affine_select(
                out=qmask[:, kt, :], in_=qmask[:, kt, :], compare_op=FP.is_ge,
                fill=0.0, base=-kt * P, pattern=[[1, QC]], channel_multiplier=-1)

        def ln_stats(x_t, ts, lnw, lnrow, lnbc, lnps):
            """mean/rstd (broadcast [128,ts] bf16) for feature-major tile
            x_t [128, CH, ts] bf16; stats over the partition(feature) dim."""
            ps_s = lnps.tile([1, ts], F32, tag="pss")
            ps_s2 = lnps.tile([1, ts], F32, tag="pss2")
            for a in range(CH):
                sq = lnw.tile([P, ts], BF16, tag="sq")
                nc.scalar.square(sq, x_t[:, a, :])
                nc.tensor.matmul(ps_s, lhsT=ones_bf, rhs=x_t[:, a, :],
                                 start=(a == 0), stop=(a == CH - 1),
                                 skip_group_check=True)
                nc.tensor.matmul(ps_s2, lhsT=ones_bf, rhs=sq,
                                 start=(a == 0), stop=(a == CH - 1),
                                 skip_group_check=True)
            mu = lnrow.tile([1, ts], F32, tag="mu")
            nc.scalar.mul(mu, ps_s, 1.0 / C)
            ex2 = lnrow.tile([1, ts], F32, tag="ex2")
            nc.scalar.mul(ex2, ps_s2, 1.0 / C)
            var = lnrow.tile([1, ts], F32, tag="var")
            nc.vector.tensor_mul(var, mu, mu)
            nc.vector.tensor_sub(var, ex2, var)
            # rstd = exp(-0.5 * ln(var + eps))
            lnv = lnrow.tile([1, ts], F32, tag="lnv")
            nc.scalar.activation(lnv, var, AF.Ln, bias=eps_sb, scale=1.0)
            rstd_bf = lnrow.tile([1, ts], BF16, tag="rstdbf")
            nc.scalar.activation(rstd_bf, lnv, AF.Exp, bias=0.0, scale=-0.5)
            mu_bf = lnrow.tile([1, ts], BF16, tag="mubf")
            nc.vector.tensor_copy(mu_bf, mu)
            mu_b = lnbc.tile([P, ts], BF16, tag="mub")
            nc.gpsimd.partition_broadcast(mu_b, mu_bf)
            rstd_b = lnbc.tile([P, ts], BF16, tag="rstdb")
            nc.gpsimd.partition_broadcast(rstd_b, rstd_bf)
            return mu_b, rstd_b

        for _rep in range(reps):
            # ---- persistent activations (outlive kvq/kto) ----
            big = tc.alloc_tile_pool(name="big", bufs=1)
            attnT = big.tile([P, CH, NQ], BF16)
            xmid = big.tile([P, CH, NQ], F32)
            xmid_bf = big.tile([P, CH, NQ], BF16)
            h2T = big.tile([P, CH, NQ], BF16)

            # ========== stage A+B: LN1 + K/V (main fp8) + Q/diag-KV (bf16) ====
            kvq = tc.alloc_tile_pool(name="kvq", bufs=1)
            if True:
                KTm = kvq.tile([P, CH, NKM], BF16)
                Vm = kvq.tile([P, NKMT // 2, 2, H, D + 1], FP8)
                KTq = kvq.tile([P, CH, NQ], BF16)
                Vq = kvq.tile([P, 2, 2, H, D + 1], FP8)    # [grp, kt, h, d+1]
                QT = kvq.tile([P, CH, NQ], BF16)

                with tc.tile_pool(name="hTp", bufs=2) as hTp, \
                     tc.tile_pool(name="hqp", bufs=1) as hqp, \
                     tc.tile_pool(name="xs", bufs=2) as xs, \
                     tc.tile_pool(name="xqp", bufs=1) as xqp, \
                     tc.tile_pool(name="lnw", bufs=3) as lnw, \
                     tc.tile_pool(name="lnrow", bufs=2) as lnrow, \
                     tc.tile_pool(name="lnbc", bufs=3) as lnbc, \
                     tc.tile_pool(name="wst", bufs=6) as wst, \
                     tc.tile_pool(name="wres", bufs=1) as wres, \
                     tc.tile_pool(name="lnps", bufs=1, space="PSUM") as lnps, \
                     tc.tile_pool(name="pb", bufs=2, space="PSUM") as pb, \
                     tc.tile_pool(name="pv", bufs=2, space="PSUM") as pv:
                    ATILES = [(0, 512), (512, 512), (1024, 512), (1536, 256)]

                    def _fetch_x(i):
                        t0, ts = ATILES[i]
                        x_t = xs.tile([P, CH, 512], BF16, tag="xt", name=f"xt{i}")
                        nc.sync.dma_start(
                            out=x_t[:, :, 0:ts], in_=xm_r[:, :, t0:t0 + ts])
                        return x_t

                    x_next = _fetch_x(0)
                    # prefetch query tokens early so the q-section LN never waits
                    xq_t = xqp.tile([P, CH, NQ], BF16, tag="xq")
                    nc.sync.dma_start(out=xq_t, in_=xq_r)
                    wv_sb = wres.tile([P, CH, C], FP8)
                    nc.gpsimd.dma_start(out=wv_sb, in_=wv8[:])
                    wk_sb = wres.tile([P, CH, C], FP8)
                    for m in range(CH):
                        nc.gpsimd.dma_start(out=wk_sb[:, :, m * P:(m + 1) * P],
                                            in_=wk8[m])
                    for i, (t0, ts) in enumerate(ATILES):
                        x_t = x_next
                        if i + 1 < len(ATILES):
                            x_next = _fetch_x(i + 1)
                        x_v = x_t[:, :, 0:ts]
                        mu_b, rstd_b = ln_stats(x_v, ts, lnw, lnrow, lnbc, lnps)
                        hT = hTp.tile([P, CH, 512], FP8, tag="hT", name=f"hT{i}")
                        for a in range(CH):
                            h0 = lnw.tile([P, 512], BF16, tag="h0")
                            nc.vector.tensor_sub(h0[:, 0:ts], x_v[:, a, :], mu_b)
                            nc.vector.tensor_mul(hT[:, a, 0:ts], h0[:, 0:ts],
                                                 rstd_b)

                        # K^T projection (fp8 DR): KT = ps/64 + bk
                        for m in range(CH):
                            ps = pb.tile([P, 512], F32, tag="bps")
                            for k in range(CH // 2):
                                nc.tensor.matmul(
                                    ps[:, 0:ts],
                                    lhsT=wk_sb[:, 2 * k:2 * k + 2, m * P:(m + 1) * P],
                                    rhs=hT[:, 2 * k:2 * k + 2, 0:ts],
                                    start=(k == 0), stop=(k == CH // 2 - 1),
                                    perf_mode=DR)
                            nc.vector.tensor_scalar(
                                out=KTm[:, m, t0:t0 + ts], in0=ps[:, 0:ts],
                                scalar1=1.0 / WS, scalar2=bk_s[:, m:m + 1],
                                op0=FP.mult, op1=FP.add)

                        # V natural (fp8 DR): per 128-key subtile; v8 = ps*8/64
                        for kk in range(ts // P):
                            kt = t0 // P + kk
                            ps = pv.tile([P, 2, 512], F32, tag="vps")
                            for k in range(CH // 2):
                                for n in range(2):
                                    nc.tensor.matmul(
                                        ps[:, n, :],
                                        lhsT=hT[:, 2 * k:2 * k + 2,
                                                kk * P:(kk + 1) * P],
                                        rhs=wv_sb[:, 2 * k:2 * k + 2,
                                                  n * 512:(n + 1) * 512],
                                        start=(k == 0), stop=(k == CH // 2 - 1),
                                        perf_mode=DR, skip_group_check=True)
                            vdst = Vm[:, kt // 2, kt % 2, :, :]
                            for n in range(2):
                                nc.vector.tensor_scalar_mul(
                                    out=vdst[:, 8 * n:8 * n + 8, 0:D],
                                    in0=ps[:, n, :].rearrange(
                                        "p (h d) -> p h d", d=D),
                                    scalar1=VS8 / WS)
                            nc.vector.memset(vdst[:, :, D:D + 1], VS8)

                    # ---- query tokens: LN + Q (bf16) + diag K/V (fp8 DR) ----
                    mu_b, rstd_b = ln_stats(xq_t, NQ, lnw, lnrow, lnbc, lnps)
                    hq = hqp.tile([P, CH, NQ], BF16, tag="hq")
                    hq8 = hqp.tile([P, CH, NQ], FP8, tag="hq8")
                    for a in range(CH):
                        nc.vector.tensor_sub(hq[:, a, :], xq_t[:, a, :], mu_b)
                        nc.vector.tensor_mul(hq[:, a, :], hq[:, a, :], rstd_b)
                        nc.gpsimd.tensor_copy(out=hq8[:, a, :], in_=hq[:, a, :])

                    for m in range(CH):
                        wt = wst.tile([P, CH, P], BF16, tag="wqt")
                        nc.sync.dma_start(out=wt, in_=wq[m])
                        ps = pb.tile([P, 512], F32, tag="bps")
                        for k in range(CH):
                            nc.tensor.matmul(ps, lhsT=wt[:, k, :], rhs=hq[:, k, :],
                                             start=(k == 0), stop=(k == CH - 1))
                        nc.vector.tensor_scalar_add(
                            out=QT[:, m, :], in0=ps, scalar1=bq_s[:, m:m + 1])
                        ps2 = pb.tile([P, 512], F32, tag="bps")
                        for k in range(CH // 2):
                            nc.tensor.matmul(
                                ps2,
                                lhsT=wk_sb[:, 2 * k:2 * k + 2, m * P:(m + 1) * P],
                                rhs=hq8[:, 2 * k:2 * k + 2, :],
                                start=(k == 0), stop=(k == CH // 2 - 1),
                                perf_mode=DR)
                        nc.vector.tensor_scalar(
                            out=KTq[:, m, :], in0=ps2,
                            scalar1=1.0 / WS, scalar2=bk_s[:, m:m + 1],
                            op0=FP.mult, op1=FP.add)
                    # diag V natural (fp8 DR), 4 key tiles of hq8
                    for kt in range(4):
                        ps = pv.tile([P, 2, 512], F32, tag="vps")
                        for k in range(CH // 2):
                            for n in range(2):
                                nc.tensor.matmul(
                                    ps[:, n, :],
                                    lhsT=hq8[:, 2 * k:2 * k + 2, kt * P:(kt + 1) * P],
                                    rhs=wv_sb[:, 2 * k:2 * k + 2,
                                              n * 512:(n + 1) * 512],
                                    start=(k == 0), stop=(k == CH // 2 - 1),
                                    perf_mode=DR, skip_group_check=True)
                        vdst = Vq[:, kt // 2, kt % 2, :, :]
                        for n in range(2):
                            nc.vector.tensor_scalar_mul(
                                out=vdst[:, 8 * n:8 * n + 8, 0:D],
                                in0=ps[:, n, :].rearrange("p (h d) -> p h d", d=D),
                                scalar1=VS8 / WS)
                        nc.vector.memset(vdst[:, :, D:D + 1], VS8)

                if stop_after == "AB":
                    with tc.tile_pool(name="dbg", bufs=2) as dbg:
                        for m in range(CH):
                            t = dbg.tile([P, NQ], F32, tag="d")
                            nc.vector.tensor_copy(out=t, in_=QT[:, m, :])
                            nc.sync.dma_start(out=yt_r[:, m, :], in_=t)
                    continue

                # odd heads' K/Q shifted to partition base 0 (matmuls with
                # base-64 operands cannot write mid-bank PSUM outputs)
                kto = tc.alloc_tile_pool(name="kto", bufs=1)
                KTo = kto.tile([D, CH, NKM + NQ], BF16)
                QTo = kto.tile([D, CH, NQ], BF16)
                for mc in range(CH):
                    nc.gpsimd.dma_start(out=KTo[:, mc, 0:NKM],
                                        in_=KTm[D:P, mc, :])
                    nc.gpsimd.dma_start(out=KTo[:, mc, NKM:NKM + NQ],
                                        in_=KTq[D:P, mc, :])
                    nc.gpsimd.dma_start(out=QTo[:, mc, :], in_=QT[D:P, mc, :])

                # ================= stage C: attention =================
                # per head pair mc: for each group g (queries g*256..):
                #   diag (2 kt fused, Vq/KTq) then prefix pairs (fp8 DR on Vm)
                with tc.tile_pool(name="att", bufs=3) as att, \
                     tc.tile_pool(name="den", bufs=2) as denp, \
                     tc.tile_pool(name="attbc", bufs=4) as attbc, \
                     tc.tile_pool(name="psy", bufs=2, space="PSUM") as psy, \
                     tc.tile_pool(name="pss", bufs=2, space="PSUM") as pss:
                    for mc in range(CH):
                        y_ps = [psy.tile([P, NQ], F32, tag=f"yps{j}",
                                         name=f"yps{j}_{mc}") for j in range(2)]
                        for g in range(2):
                            qcols = slice(g * QC, (g + 1) * QC)
                            # --- diagonal: 2 kt fused ---
                            ps_s = pss.tile([P, 2, 2, QC], F32, tag="sps")
                            for kt in range(2):
                                dcol = (2 * g + kt) * P
                                nc.tensor.matmul(
                                    ps_s[:, kt, 0, :],
                                    lhsT=KTq[0:D, mc, dcol:dcol + P],
                                    rhs=QT[0:D, mc, qcols],
                                    start=True, stop=True,
                                    skip_group_check=True)
                                nc.tensor.matmul(
                                    ps_s[:, kt, 1, :],
                                    lhsT=KTo[:, mc, NKM + dcol:NKM + dcol + P],
                                    rhs=QTo[:, mc, qcols],
                                    start=True, stop=True,
                                    skip_group_check=True)
                            p_t = att.tile([P, 2, 2, QC], BF16, tag="ptd")
                            nc.scalar.activation(
                                out=p_t, in_=ps_s, func=AF.Exp,
                                bias=kb_sb[:, g * (GPAIRS[0] + 1):
                                           g * (GPAIRS[0] + 1) + 1],
                                scale=SM_SCALE)
                            pm = att.tile([P, 2, 2, QC], FP8, tag="pm")
                            nc.vector.tensor_mul(
                                pm, p_t,
                                qmask[:].unsqueeze(2).broadcast_to(
                                    (P, 2, 2, QC)))
                            for j in range(2):
                                nc.tensor.matmul(
                                    y_ps[j][0:D + 1, qcols],
                                    lhsT=Vq[:, g, :, 2 * mc + j, :],
                                    rhs=pm[:, :, j, :],
                                    start=True, stop=False,
                                    perf_mode=DR, skip_group_check=True)
                            # --- prefix pairs ---
                            npair = GPAIRS[g]
                            for t in range(npair):
                                ps_p = pss.tile([P, 2, 2, QC], F32, tag="sps")
                                for kt in range(2):
                                    mcol = (2 * t + kt) * P
                                    nc.tensor.matmul(
                                        ps_p[:, kt, 0, :],
                                        lhsT=KTm[0:D, mc, mcol:mcol + P],
                                        rhs=QT[0:D, mc, qcols],
                                        start=True, stop=True,
                                        skip_group_check=True)
                                    nc.tensor.matmul(
                                        ps_p[:, kt, 1, :],
                                        lhsT=KTo[:, mc, mcol:mcol + P],
                                        rhs=QTo[:, mc, qcols],
                                        start=True, stop=True,
                                        skip_group_check=True)
                                p_p = att.tile([P, 2, 2, QC], FP8, tag="pt")
                                nc.scalar.activation(
                                    out=p_p, in_=ps_p, func=AF.Exp,
                                    bias=kb_sb[:, g * (GPAIRS[0] + 1) + 1 + t:
                                               g * (GPAIRS[0] + 1) + 2 + t],
                                    scale=SM_SCALE)
                                for j in range(2):
                                    nc.tensor.matmul(
                                        y_ps[j][0:D + 1, qcols],
                                        lhsT=Vm[:, t, :, 2 * mc + j, :],
                                        rhs=p_p[:, :, j, :],
                                        start=False, stop=(t == npair - 1),
                                        perf_mode=DR, skip_group_check=True)
                        # epilogue: drain y_ps fast (DVE copy), then
                        # rec = exp(-ln(den)) and normalize off-critical-path
                        den2 = denp.tile([1, 2, NQ], F32, tag="den")
                        ya = denp.tile([D, 2, NQ], F32, tag="ya")
                        for j in range(2):
                            nc.vector.tensor_copy(out=den2[:, j, :],
                                                  in_=y_ps[j][D:D + 1, :])
                            nc.vector.tensor_copy(out=ya[:, j, :],
                                                  in_=y_ps[j][0:D, :])
                        lden = denp.tile([1, 2, NQ], F32, tag="lden")
                        nc.scalar.activation(lden, den2, AF.Ln, bias=0.0,
                                             scale=1.0)
                        rec2 = denp.tile([1, 2, NQ], F32, tag="rec")
                        nc.scalar.activation(rec2, lden, AF.Exp, bias=0.0,
                                             scale=-1.0)
                        for j in range(2):
                            rec_b = attbc.tile([D, NQ], F32, tag="recb")
                            nc.gpsimd.partition_broadcast(rec_b, rec2[:, j, :])
                            nc.vector.tensor_mul(
                                attnT[j * D:(j + 1) * D, mc, :],
                                ya[:, j, :], rec_b)

                if stop_after == "C":
                    with tc.tile_pool(name="dbg", bufs=2) as dbg:
                        for m in range(CH):
                            t = dbg.tile([P, NQ], F32, tag="d")
                            nc.vector.tensor_copy(out=t, in_=attnT[:, m, :])
                            nc.sync.dma_start(out=yt_r[:, m, :], in_=t)
                    big.release()
                    continue

                kto.release()
                kvq.release()

                # ================= stage D: out-proj + residual =================
                with tc.tile_pool(name="wst2", bufs=6) as wst2, \
                     tc.tile_pool(name="xqs", bufs=3) as xqs, \
                     tc.tile_pool(name="pd", bufs=3, space="PSUM") as pd:
                    for m in range(CH):
                        wt = wst2.tile([P, CH, P], BF16, tag="wot")
                        nc.sync.dma_start(out=wt, in_=wo[m])
                        ps = pd.tile([P, NQ], F32, tag="dps")
                        for k in range(CH):
                            nc.tensor.matmul(ps, lhsT=wt[:, k, :], rhs=attnT[:, k, :],
                                             start=(k == 0), stop=(k == CH - 1))
                        xq_t = xqs.tile([P, NQ], F32, tag="xq")
                        nc.sync.dma_start(out=xq_t, in_=xqf_r[:, m, :])
                        nc.vector.scalar_tensor_tensor(
                            out=xmid[:, m, :], in0=ps, scalar=bo_s[:, m:m + 1],
                            in1=xq_t, op0=FP.add, op1=FP.add)
                        nc.gpsimd.tensor_copy(out=xmid_bf[:, m, :],
                                              in_=xmid[:, m, :])

                # ================= stage E: LN2 =================
                with tc.tile_pool(name="lnw2", bufs=3) as lnw, \
                     tc.tile_pool(name="lnrow2", bufs=2) as lnrow, \
                     tc.tile_pool(name="lnbc2", bufs=3) as lnbc, \
                     tc.tile_pool(name="lnps2", bufs=2, space="PSUM") as lnps:
                    for t in range(2):
                        t0 = t * QC
                        x_t = xmid_bf[:, :, t0:t0 + QC]
                        mu_b, rstd_b = ln_stats(x_t, QC, lnw, lnrow, lnbc, lnps)
                        for a in range(CH):
                            nc.vector.tensor_sub(h2T[:, a, t0:t0 + QC],
                                                 x_t[:, a, :], mu_b)
                            nc.vector.tensor_mul(h2T[:, a, t0:t0 + QC],
                                                 h2T[:, a, t0:t0 + QC], rstd_b)

                if stop_after == "E":
                    with tc.tile_pool(name="dbg", bufs=2) as dbg:
                        for m in range(CH):
                            t = dbg.tile([P, NQ], F32, tag="d")
                            nc.vector.tensor_copy(out=t, in_=h2T[:, m, :])
                            nc.sync.dma_start(out=yt_r[:, m, :], in_=t)
                    big.release()
                    continue

                # ================= stage F: MLP up + GELU =================
                with tc.tile_pool(name="gp", bufs=1) as gp:
                    g_sb = gp.tile([P, FH, NQ], FP8)
                    with tc.tile_pool(name="w1s", bufs=6) as w1s, \
                         tc.tile_pool(name="pf", bufs=3, space="PSUM") as pf:
                        for m in range(FH):
                            wt = w1s.tile([P, CH, P], BF16, tag="w1t")
                            nc.sync.dma_start(out=wt, in_=w1[m])
                            ps = pf.tile([P, NQ], F32, tag="fps")
                            for k in range(CH):
                                nc.tensor.matmul(ps, lhsT=wt[:, k, :],
                                                 rhs=h2T[:, k, :],
                                                 start=(k == 0), stop=(k == CH - 1))
                            nc.scalar.activation(out=g_sb[:, m, :], in_=ps,
                                                 func=AF.Gelu,
                                                 bias=b1_sb[:, m:m + 1], scale=1.0)

                    # ============= stage G: MLP down + residual =============
                    with tc.tile_pool(name="w2s", bufs=3) as w2s, \
                         tc.tile_pool(name="outs", bufs=3) as outs, \
                         tc.tile_pool(name="pg", bufs=3, space="PSUM") as pg:
                        for m in range(CH):
                            wt = w2s.tile([P, FH, P], FP8, tag="w2t")
                            nc.sync.dma_start(out=wt, in_=w2[m])
                            ps = pg.tile([P, NQ], F32, tag="gps")
                            for k in range(FH // 2):
                                nc.tensor.matmul(
                                    ps, lhsT=wt[:, 2 * k:2 * k + 2, :],
                                    rhs=g_sb[:, 2 * k:2 * k + 2, :],
                                    start=(k == 0), stop=(k == FH // 2 - 1),
                                    perf_mode=DR)
                            out_t = outs.tile([P, NQ], F32, tag="ot")
                            nc.vector.tensor_scalar(
                                out=out_t, in0=ps, scalar1=1.0 / WS,
                                scalar2=b2_s[:, m:m + 1],
                                op0=FP.mult, op1=FP.add)
                            nc.vector.tensor_add(out_t, out_t, xmid[:, m, :])
                            nc.sync.dma_start(out=yt_r[:, m, :], in_=out_t)

                big.release()

    nc.compile()
    return nc


def _prep_weight(w, mtiles):
    """[Cin, Cout] -> [mtiles, 128, Cin] tile-contiguous (lhsT layout)."""
    cin, cout = w.shape
    a = cin // P
    r = w.reshape(a, P, mtiles, P).transpose(2, 1, 0, 3).reshape(mtiles, P, a * P)
    return np.ascontiguousarray(r)


def _col_table(*vecs):
    cols = [v.reshape(-1, P).T for v in vecs]
    return np.ascontiguousarray(np.concatenate(cols, axis=1)).astype(np.float32)


def prepare_in_maps(x, ln1_g, ln1_b, wq, bq, wk, bk, wv, bv, wo, bo,
                    ln2_g, ln2_b, w1, b1, w2, b2):
    f = np.float32
    x = np.asarray(x, f)
    ln1_g, ln1_b = np.asarray(ln1_g, f), np.asarray(ln1_b, f)
    ln2_g, ln2_b = np.asarray(ln2_g, f), np.asarray(ln2_b, f)
    wq, wk, wv, wo = (np.asarray(w, f) for w in (wq, wk, wv, wo))
    w1, w2 = np.asarray(w1, f), np.asarray(w2, f)
    bq, bk, bv, bo = (np.asarray(b, f) for b in (bq, bk, bv, bo))
    b1, b2 = np.asarray(b1, f), np.asarray(b2, f)

    # fold layernorm affine into downstream weights; fold bv into bo
    wq_f = ln1_g[:, None] * wq
    wk_f = ln1_g[:, None] * wk
    wv_f = ln1_g[:, None] * wv
    w1_f = ln2_g[:, None] * w1
    bq_f = bq + ln1_b @ wq
    bk_f = bk + ln1_b @ wk
    bv_f = bv + ln1_b @ wv
    b1_f = b1 + ln2_b @ w1
    bo_f = bo + bv_f @ wo

    bf = ml_dtypes.bfloat16
    f8 = ml_dtypes.float8_e4m3
    wq_p = _prep_weight(wq_f, CH).astype(bf)
    wk8 = (_prep_weight(wk_f, CH) * WS).astype(f8)
    wo_p = _prep_weight(wo, CH).astype(bf)
    w1_p = _prep_weight(w1_f, FH).astype(bf)
    w2_p = (_prep_weight(w2, CH) * WS).astype(f8)
    # natural-V rhs layout [128, CH, C]
    wv8 = np.ascontiguousarray(
        wv_f.reshape(CH, P, C).transpose(1, 0, 2) * WS).astype(f8)
    bias_tab = _col_table(bq_f, bk_f, bo_f, b2)
    b1_tab = np.ascontiguousarray(b1_f.reshape(FH, P).T)

    in_maps = []
    for c in range(NCORES):
        b, j = divmod(c, 4)
        xb = x[b]
        xm_i = np.ascontiguousarray(xb[0:NKM].T)
        qrows = np.concatenate([xb[j * QC:(j + 1) * QC],
                                xb[(7 - j) * QC:(8 - j) * QC]], 0)
        xq_i = np.ascontiguousarray(qrows.T)
        # kbias entries: [g0 diag, g0 pairs(3), g1 diag, g1 pairs(7)]
        kb = np.full((NPAIR_BIAS,), NEG, f)
        kb[0] = PB
        for t in range(GPAIRS[0]):
            if t < j:
                kb[1 + t] = PB
        kb[1 + GPAIRS[0]] = PB
        for t in range(GPAIRS[1]):
            if t < 7 - j:
                kb[2 + GPAIRS[0] + t] = PB
        kb_i = np.ascontiguousarray(np.broadcast_to(kb, (P, NPAIR_BIAS)))
        in_maps.append({
            "xm": xm_i.astype(bf), "xq": xq_i.astype(bf), "xqf": xq_i,
            "wq": wq_p, "wo": wo_p, "w1": w1_p, "w2": w2_p,
            "wk8": wk8, "wv8": wv8,
            "bias": bias_tab, "b1": b1_tab, "kbias": kb_i,
        })
    return in_maps


def assemble_output(per_core_yt):
    out = np.empty((B, T, C), np.float32)
    for c in range(NCORES):
        b, j = divmod(c, 4)
        yt_i = np.asarray(per_core_yt[c])
        out[b, j * QC:(j + 1) * QC] = yt_i[:, 0:QC].T
        out[b, (7 - j) * QC:(8 - j) * QC] = yt_i[:, QC:NQ].T
    return out


def kernel(**inputs):
    if "nc" not in _NC_CACHE:
        _NC_CACHE["nc"] = _build_nc()
    nc = _NC_CACHE["nc"]
    in_maps = prepare_in_maps(**inputs)
    res = run_bass_kernel_spmd(nc, in_maps, core_ids=list(range(NCORES)))
    return assemble_output([res.results[c]["yt"] for c in range(NCORES)])
